# revision 24
# baseline (speedup 1.0000x reference)
"""Multi-head attention with bias, distributed over 8 trn2 NeuronCores.

Reference computation (per batch b):
    q = (x @ Wq.T) * depth**-0.5 ; k = y @ Wk.T ; v = y @ Wv.T     (per-head split)
    out = softmax(q @ k.T + bias) @ v @ Wo.T

Sharding v2 (tensor-parallel over heads): 8 cores = 4 batches x 2
head-groups of 8 heads.  Core c handles batch b = c//2 and heads
(c%2)*8 .. +8, over the FULL query sequence.  Wq/Wk/Wv are column-split
(by head) and Wo row-split; each core returns a partial output
[S, D] and the HOST sums the two partials per batch (the "all-reduce").
No redundant k/v projection work, no device collectives.

Device-side layout (feature dim on partitions):
    qT/kT = W.T-projected activations [dg=512, S]; head pair t lives on
    partitions of tile t.  v natural [kk, head, 97] with layout
    [0]*31 | 1 | v(64) | 1  so that
      even heads: lhsT cols 32:97 -> psum rows 0:64 attn + row 64 denom
      odd  heads: lhsT cols 0:96, base_partition 32 -> psum row 63 denom
                  + rows 64:128 attn  (legal PE out bases are {0,32,64})
    logitsT[kk, i] = kT-slice.T @ qT-slice  (K=64)
    expw = exp(logitsT) * exp(bias).T      (exp(bias) precomputed on host)
    attnT(+denom row) = [v|1].T @ expw     (K=128, denom rides along)
    normalization: DVE reciprocal of the denom row, PE ones-matmul
    broadcasts it across 64 partitions, DVE multiply writes normalized
    attn straight into SBUF in the out-projection layout.  outT = Wo.T
    partial projection, DMA'd directly from PSUM (f32).
Host does: transposes, bf16 casts, exp(bias), scale fold into Wq,
and the final pairwise partial sum.
"""

import numpy as np
import ml_dtypes
from contextlib import ExitStack

import concourse.bass as bass
import concourse.mybir as mybir
import concourse.tile as tile
from concourse import bacc
from concourse.bass_utils import run_bass_kernel_spmd

# full-problem dims (hardcoded per spec)
B, S, D, H = 4, 2048, 1024, 16
DEPTH = D // H            # 64
P = 128
NCORES = 8
HG = H // 2               # heads per core = 8
DG = HG * DEPTH           # feature dims per core = 512

BF = mybir.dt.bfloat16
F32 = mybir.dt.float32
EXP = mybir.ActivationFunctionType.Exp

TRACE = False
PACK_LOGITS = True        # v3: tile_position row-packing of K=64 logits mms
last_exec_time_ns = None
last_results = None


def _attn_body(ctx, tc, io):
    nc = tc.nc
    S_ = S                 # kv/q seq len (full)
    NT = D // P            # 8 input-dim tiles
    KT = S_ // P           # 16 kk tiles
    NDT = DG // P          # 4 head-pair tiles
    CW = 512               # free-dim chunk width
    NICH = S_ // CW        # 4 query chunks
    xT, yT, ebT, wqT, wkT, wvT, woT, outT = (
        io[k] for k in ("xT", "yT", "ebT", "wqT", "wkT", "wvT", "woT", "outT"))

    # ---- persistent pools (live through the whole kernel) ----
    qpool = ctx.enter_context(tc.tile_pool(name="qpool", bufs=NDT))
    kpool = ctx.enter_context(tc.tile_pool(name="kpool", bufs=NDT))
    vpool = ctx.enter_context(tc.tile_pool(name="vpool", bufs=KT))
    wopool = ctx.enter_context(tc.tile_pool(name="wopool", bufs=NDT))
    anpool = ctx.enter_context(tc.tile_pool(name="anpool", bufs=NDT))

    q_sb = [qpool.tile([P, S_], BF, tag="qT", name=f"q{t}", bufs=NDT)
            for t in range(NDT)]
    k_sb = [kpool.tile([P, S_], BF, tag="kT", name=f"k{t}", bufs=NDT)
            for t in range(NDT)]
    v_sb = [vpool.tile([P, HG, 66], BF, tag="v66", name=f"v{c}", bufs=KT)
            for c in range(KT)]
    wo_sb = [wopool.tile([P, D], BF, tag="wo", name=f"wo{t}", bufs=NDT)
             for t in range(NDT)]
    an_sb = [anpool.tile([P, S_], BF, tag="an", name=f"an{t}", bufs=NDT)
             for t in range(NDT)]

    # ---- psum: plp 4 banks now; pap/pbc (4 banks) allocated after the
    # prologue's ppj pool (2 banks) closes so peak stays at 8 banks ----
    plp = ctx.enter_context(tc.tile_pool(name="plp", bufs=2, space="PSUM"))

    # ================= prologue: load + q/k/v projections ==============
    with tc.tile_pool(name="xpool", bufs=NT) as xpool, \
         tc.tile_pool(name="ypool", bufs=NT) as ypool, \
         tc.tile_pool(name="wqpool", bufs=NT) as wqpool, \
         tc.tile_pool(name="wkpool", bufs=NT) as wkpool, \
         tc.tile_pool(name="wvpool", bufs=NT) as wvpool, \
         tc.tile_pool(name="ppj", bufs=2, space="PSUM") as ppj:
        x_sb = [xpool.tile([P, S_], BF, tag="xT", name=f"x{t}", bufs=NT)
                for t in range(NT)]
        wq_sb = [wqpool.tile([P, DG], BF, tag="wq", name=f"wq{t}", bufs=NT)
                 for t in range(NT)]
        y_sb = [ypool.tile([P, S_], BF, tag="yT", name=f"y{t}", bufs=NT)
                for t in range(NT)]
        wk_sb = [wkpool.tile([P, DG], BF, tag="wk", name=f"wk{t}", bufs=NT)
                 for t in range(NT)]
        wv_sb = [wvpool.tile([P, DG], BF, tag="wv", name=f"wv{t}", bufs=NT)
                 for t in range(NT)]
        for t in range(NT):
            nc.sync.dma_start(out=x_sb[t], in_=xT[t * P:(t + 1) * P, :])
            nc.sync.dma_start(out=wq_sb[t], in_=wqT[t * P:(t + 1) * P, :])
            nc.sync.dma_start(out=y_sb[t], in_=yT[t * P:(t + 1) * P, :])
            nc.sync.dma_start(out=wk_sb[t], in_=wkT[t * P:(t + 1) * P, :])
            nc.sync.dma_start(out=wv_sb[t], in_=wvT[t * P:(t + 1) * P, :])
        for t in range(NDT):
            nc.gpsimd.dma_start(out=wo_sb[t], in_=woT[t * P:(t + 1) * P, :])

        # warm-up heartbeats: tiny matmuls chained to arriving input DMAs
        # keep the PE HAM activity window alive through the load phase
        jnk0 = plp.tile([P, 1024], F32, tag="pl", name="jnk0", bufs=2)
        for t in range(NT):
            nc.tensor.matmul(jnk0[0:1, 0:CW], lhsT=x_sb[t][0:1, 0:1],
                             rhs=x_sb[t][0:1, 0:CW], start=True, stop=True)
            nc.tensor.matmul(jnk0[0:1, 0:CW], lhsT=y_sb[t][0:1, 0:1],
                             rhs=y_sb[t][0:1, 0:CW], start=True, stop=True)

        # interleaved q/k/v projections: one task per (kind, chunk), all
        # accumulating over u as the input DMAs land
        ptasks = []
        for td in range(NDT):
            for n0 in range(0, S_, CW):
                ptasks.append(("q", td, n0))
                ptasks.append(("k", td, n0))
        for c in range(KT):
            ptasks.append(("v", c, 0))
        order = []
        qk = [p for p in ptasks if p[0] != "v"]
        vv = [p for p in ptasks if p[0] == "v"]
        for idx in range(len(qk)):
            order.append(qk[idx])
            if idx % 2 == 1:
                order.append(vv[idx // 2])
        assert len(order) == len(ptasks)
        for kind, a, n0 in order:
            ps = ppj.tile([P, CW], F32, tag="pj", name=f"p{kind}{a}_{n0}",
                          bufs=2)
            if kind == "q":
                for u in range(NT):
                    nc.tensor.matmul(ps,
                                     lhsT=wq_sb[u][:, a * P:(a + 1) * P],
                                     rhs=x_sb[u][:, n0:n0 + CW],
                                     start=(u == 0), stop=(u == NT - 1))
                nc.vector.tensor_copy(q_sb[a][:, n0:n0 + CW], ps)
            elif kind == "k":
                for u in range(NT):
                    nc.tensor.matmul(ps,
                                     lhsT=wk_sb[u][:, a * P:(a + 1) * P],
                                     rhs=y_sb[u][:, n0:n0 + CW],
                                     start=(u == 0), stop=(u == NT - 1))
                nc.vector.tensor_copy(k_sb[a][:, n0:n0 + CW], ps)
            else:
                vt = v_sb[a]
                nc.vector.memset(vt[:, :, 64:65], 1.0)
                for u in range(NT):
                    nc.tensor.matmul(ps,
                                     lhsT=y_sb[u][:, a * P:(a + 1) * P],
                                     rhs=wv_sb[u][:, 0:DG],
                                     start=(u == 0), stop=(u == NT - 1))
                nc.vector.tensor_copy(
                    vt[:, :, 0:64], ps.rearrange("p (h d) -> p h d", d=DEPTH))

    # eb tiles stream in now (first use is ~immediately below; DMA of
    # tile c completes well before the c-loop consumes it)
    ebpool = ctx.enter_context(tc.tile_pool(name="ebpool", bufs=KT))
    eb_sb = [ebpool.tile([P, S_], BF, tag="eb", name=f"eb{c}", bufs=KT)
             for c in range(KT)]
    for c in range(KT):
        nc.sync.dma_start(out=eb_sb[c], in_=ebT[c * P:(c + 1) * P, :])

    ewpool = ctx.enter_context(tc.tile_pool(name="ewpool", bufs=3))
    ew2pool = ctx.enter_context(tc.tile_pool(name="ew2pool", bufs=3))
    recpool = ctx.enter_context(tc.tile_pool(name="recpool", bufs=2))
    dpool = ctx.enter_context(tc.tile_pool(name="dpool", bufs=2,
                                           space="DRAM"))
    pap = ctx.enter_context(tc.tile_pool(name="pap", bufs=4, space="PSUM"))

    # ================= main loop =================
    # Fully flattened software pipeline over (ich, t, c): the attnV stream
    # runs LOOK steps behind the logits stream and never drains at pair or
    # chunk boundaries.  Normalization (DVE/gpsimd/DMA) and out-projection
    # are scheduled as step-indexed events so the PE never blocks on them.
    LOOK = 3

    def norm_thunks(ich, t, pst):
        """Normalization as a list of small thunks, one DVE op each, so
        they interleave with the steady ew2-mul stream without bursts."""
        isl = slice(ich * CW, (ich + 1) * CW)
        pa, pb = pst["pa"], pst["pb"]
        dab = recpool.tile([1, 2 * CW], F32, tag="dab", name=f"dab{t}{ich}",
                           bufs=2)
        raf = recpool.tile([1, 2 * CW], F32, tag="raf", name=f"raf{t}{ich}",
                           bufs=2)
        rab = recpool.tile([1, 2 * CW], BF, tag="rab", name=f"rab{t}{ich}",
                           bufs=2)
        rdr = dpool.tile([1, 2 * CW], BF, tag="rdr", name=f"rdr{t}{ich}",
                         bufs=2)
        bcs = recpool.tile([64, 2 * CW], BF, tag="bcs", name=f"bcs{t}{ich}",
                           bufs=2)
        anb = recpool.tile([64, CW], BF, tag="anb", name=f"anb{t}{ich}",
                           bufs=2)

        def s1():
            nc.vector.tensor_copy(dab[:, 0:CW], pa[64:65, :])

        def s2():
            nc.vector.tensor_copy(dab[:, CW:2 * CW], pb[64:65, :])

        def s3():
            nc.vector.reciprocal_approx_fast(raf, dab)
            nc.vector.tensor_copy(rab, raf)
            nc.sync.dma_start(out=rdr, in_=rab)

        def s4():
            nc.sync.dma_start(out=bcs[:, 0:CW],
                              in_=rdr[0:1, 0:CW].partition_broadcast(64))
            nc.sync.dma_start(out=bcs[:, CW:2 * CW],
                              in_=rdr[0:1, CW:2 * CW].partition_broadcast(64))

        def s5():
            nc.vector.tensor_mul(an_sb[t][0:64, isl], pa[0:64, :],
                                 bcs[:, 0:CW])

        def s6():
            nc.vector.tensor_mul(anb, pb[0:64, :], bcs[:, CW:2 * CW])
            nc.sync.dma_start(out=an_sb[t][64:128, isl], in_=anb)

        return [s1, s2, s3, s4, s5, s6]

    def emit_opm(ich, m):
        """One m-tile of the partial output projection for chunk ich."""
        isl = slice(ich * CW, (ich + 1) * CW)
        po = pap.tile([P, CW], F32, tag="pattn", name=f"po{m}_{ich}",
                      bufs=4)
        for kt in range(NDT):
            nc.tensor.matmul(po,
                             lhsT=wo_sb[kt][:, m * P:(m + 1) * P],
                             rhs=an_sb[kt][:, isl],
                             start=(kt == 0), stop=(kt == NDT - 1))
        osb = ew2pool.tile([P, CW], F32, tag="osb", name=f"o{m}_{ich}",
                           bufs=2)
        nc.vector.tensor_copy(osb, po)
        nc.sync.dma_start(out=outT[m * P:(m + 1) * P, isl], in_=osb)

    def emit_L(ich, t, c, ew2s):
        isl = slice(ich * CW, (ich + 1) * CW)
        plt = plp.tile([P, 1024], F32, tag="pl", name=f"pl{t}_{c}", bufs=2)
        kw = dict(start=True, stop=True)
        nc.tensor.matmul(plt[:, 0:CW],
                         lhsT=k_sb[t][0:64, c * P:(c + 1) * P],
                         rhs=q_sb[t][0:64, isl],
                         tile_position=(0, 0) if PACK_LOGITS else None,
                         **kw)
        nc.tensor.matmul(plt[:, CW:2 * CW],
                         lhsT=k_sb[t][64:128, c * P:(c + 1) * P],
                         rhs=q_sb[t][64:128, isl],
                         tile_position=(64, 0) if PACK_LOGITS else None,
                         **kw)
        ew = ewpool.tile([P, 1024], BF, tag="ew", name=f"ew{t}_{c}", bufs=3)
        nc.scalar.activation(ew, plt, EXP)
        ew2 = ew2pool.tile([P, 1024], BF, tag="ew2", name=f"ew2{t}_{c}",
                           bufs=4)
        ebb = eb_sb[c][:, isl].unsqueeze(1).broadcast_to([P, 2, CW])
        nc.vector.tensor_mul(ew2.rearrange("p (a b) -> p a b", a=2),
                             ew.rearrange("p (a b) -> p a b", a=2), ebb)
        ew2s[c] = ew2

    def emit_A(st):
        ich, t, c = st["key"]
        ew2 = st["ew2s"][c]
        kw = dict(start=(c == 0), stop=(c == KT - 1))
        nc.tensor.matmul(st["pa"][0:65, :], lhsT=v_sb[c][:, 2 * t, 0:65],
                         rhs=ew2[:, 0:CW], **kw)
        nc.tensor.matmul(st["pb"][0:65, :], lhsT=v_sb[c][:, 2 * t + 1, 0:65],
                         rhs=ew2[:, CW:2 * CW], **kw)

    steps = [(ich, t, c) for ich in range(NICH) for t in range(NDT)
             for c in range(KT)]
    events = {}            # step index -> list of thunks
    pair_state = {}        # (ich, t) -> state dict
    astream = []           # per-step A state refs, parallel to steps

    def do_L(i):
        ich, t, c = steps[i]
        if c == 0:
            pa = pap.tile([P, CW], F32, tag="pattn", name=f"pa{t}_{ich}",
                          bufs=4)
            pb = pap.tile([P, CW], F32, tag="pattn", name=f"pb{t}_{ich}",
                          bufs=4)
            pair_state[(ich, t)] = {"key": None, "pa": pa, "pb": pb,
                                    "ew2s": {}}
        st = pair_state[(ich, t)]
        st = dict(st, key=(ich, t, c))
        emit_L(ich, t, c, st["ew2s"])
        astream.append(st)

    def do_A(j, i):
        emit_A(astream[j])
        jich, jt, jc = steps[j]
        if jc == KT - 1:
            # pair (jich, jt) attn complete: schedule its normalization
            pst = pair_state[(jich, jt)]

            for off, th in enumerate(norm_thunks(jich, jt, pst)):
                events.setdefault(i + 1 + off, []).append(th)
            if jt == NDT - 1:
                for m in range(NT):
                    events.setdefault(i + 8 + m, []).append(
                        lambda jich=jich, m=m: emit_opm(jich, m))

    n = len(steps)
    for i in range(0, n, 2):
        # L/A emitted in pairs to halve PE tiling-mode switches
        do_L(i)
        if i + 1 < n:
            do_L(i + 1)
        for j in (i - LOOK, i - LOOK + 1):
            if 0 <= j < n - 0 and j <= i - LOOK + 1:
                if j >= 0:
                    do_A(j, i)
        for ii in (i, i + 1):
            for th in events.pop(ii, ()):
                th()

    # tail: trailing A steps, then the last pair's norm overlapped with a
    # partially-accumulated final out-projection (kt 0..2 first, kt 3 last)
    for i in sorted(events):
        for th in events[i]:
            th()
    for j in range(n - LOOK, n):
        emit_A(astream[j])
    lich, lt = NICH - 1, NDT - 1
    pst = pair_state[(lich, lt)]
    thunks = norm_thunks(lich, lt, pst)
    lisl = slice(lich * CW, (lich + 1) * CW)
    pos = []
    for m in range(2):
        po = pap.tile([P, CW], F32, tag="pattn", name=f"pot{m}", bufs=4)
        pos.append(po)
    for th in thunks[:4]:
        th()
    for m in range(2):
        for kt in range(NDT - 1):
            nc.tensor.matmul(pos[m], lhsT=wo_sb[kt][:, m * P:(m + 1) * P],
                             rhs=an_sb[kt][:, lisl],
                             start=(kt == 0), stop=False)
    for th in thunks[4:]:
        th()
    for m in range(2):
        nc.tensor.matmul(pos[m], lhsT=wo_sb[NDT - 1][:, m * P:(m + 1) * P],
                         rhs=an_sb[NDT - 1][:, lisl],
                         start=False, stop=True)
        osb = ew2pool.tile([P, CW], F32, tag="osb", name=f"ot{m}", bufs=2)
        nc.vector.tensor_copy(osb, pos[m])
        nc.sync.dma_start(out=outT[m * P:(m + 1) * P, lisl], in_=osb)
    for m in range(2, NT):
        emit_opm(lich, m)


def build_nc():
    nc = bacc.Bacc("TRN2", target_bir_lowering=False, debug=False)
    io = {
        "xT": nc.dram_tensor("xT", [D, S], BF, kind="ExternalInput").ap(),
        "yT": nc.dram_tensor("yT", [D, S], BF, kind="ExternalInput").ap(),
        "ebT": nc.dram_tensor("ebT", [S, S], BF, kind="ExternalInput").ap(),
        "wqT": nc.dram_tensor("wqT", [D, DG], BF, kind="ExternalInput").ap(),
        "wkT": nc.dram_tensor("wkT", [D, DG], BF, kind="ExternalInput").ap(),
        "wvT": nc.dram_tensor("wvT", [D, DG], BF, kind="ExternalInput").ap(),
        "woT": nc.dram_tensor("woT", [DG, D], BF, kind="ExternalInput").ap(),
        "outT": nc.dram_tensor("outT", [D, S], F32,
                               kind="ExternalOutput").ap(),
    }
    with tile.TileContext(nc) as tc:
        with ExitStack() as ctx:
            _attn_body(ctx, tc, io)
    nc.compile()
    return nc


_NC_CACHE = None


def kernel(x, y, bias, Wq, Wk, Wv, Wo):
    global _NC_CACHE, last_exec_time_ns, last_results
    x = np.asarray(x, np.float32)
    y = np.asarray(y, np.float32)
    bias = np.asarray(bias, np.float32)
    Wq, Wk, Wv, Wo = (np.asarray(w, np.float32) for w in (Wq, Wk, Wv, Wo))
    if _NC_CACHE is None:
        _NC_CACHE = build_nc()
    nc = _NC_CACHE

    bf = ml_dtypes.bfloat16
    scale = DEPTH ** -0.5
    eb = np.exp(bias[0, 0].astype(np.float32))
    ebT = np.ascontiguousarray(eb.T).astype(bf)
    xT_all = [np.ascontiguousarray(x[b].T).astype(bf) for b in range(B)]
    yT_all = [np.ascontiguousarray(y[b].T).astype(bf) for b in range(B)]
    wqT_g, wkT_g, wvT_g, woT_g = [], [], [], []
    for g in range(2):
        rows = slice(g * DG, (g + 1) * DG)
        wqT_g.append(np.ascontiguousarray(Wq[rows, :].T * scale).astype(bf))
        wkT_g.append(np.ascontiguousarray(Wk[rows, :].T).astype(bf))
        wvT_g.append(np.ascontiguousarray(Wv[rows, :].T).astype(bf))
        woT_g.append(np.ascontiguousarray(Wo[:, rows].T).astype(bf))

    in_maps = []
    for core in range(NCORES):
        b, g = divmod(core, 2)
        in_maps.append({
            "xT": xT_all[b], "yT": yT_all[b], "ebT": ebT,
            "wqT": wqT_g[g], "wkT": wkT_g[g], "wvT": wvT_g[g],
            "woT": woT_g[g],
        })

    res = run_bass_kernel_spmd(nc, in_maps, core_ids=list(range(NCORES)),
                               trace=TRACE)
    last_exec_time_ns = res.exec_time_ns
    last_results = res
    out = np.empty((B, S, D), np.float32)
    for b in range(B):
        out[b] = (res.results[2 * b]["outT"].T.astype(np.float32)
                  + res.results[2 * b + 1]["outT"].T.astype(np.float32))
    return out
